# revision 12
# baseline (speedup 1.0000x reference)
"""Multi-head attention (B=2, N=4096, D=768, H=8) on 8 trn2 NeuronCores.

Sharding: core c handles batch b = c//4 and head-pair hp = c%4 (heads 2hp,
2hp+1).  Each core computes the qkv projection for its 2 heads plus full
4096x4096 attention for them; no cross-core communication.

Device-side layout (per core):
  xT    [768, 4096] fp16   x[b] transposed (host-prepped)
  wqk   [768, 384]  fp16   [Wq_h0 | Wq_h1 | Wk_h0 | Wk_h1]  (UNSCALED)
  bqk   [64, 8]     fp32   bias halves: col 2j=b_j[0:64], col 2j+1=b_j[64:96]
  wv    [768, 194]  fp16   [Wv_h0 | 0 | Wv_h1 | 0]
  wvaug [1, 194]    fp16   [bv_h0 | 1 | bv_h1 | 1]  (ones row of aug x)
  out   [2, 8, 128, 388] fp32 unnormalized numerators + denominators

Algorithm: q/k are produced at natural scale (softmax SCALE applied inside
the exp) and stored as fp8e4m3 in DoubleRow layout [64, 2, N] (dh=96 zero-padded
to 128 and split as 64 partitions x 2 interleaved k-subtiles).  Scores run on
the PE in fp8 DoubleRow perf mode at 0.5 cycles/row -- half the fp16 cost.
V stays fp16 (fp8 V would blow the 2e-2 error budget).

Per query window of 512 tokens, scores are computed transposed
S^T[m, n] = kT8.T @ qT8 one 128-key tile at a time; PAIRS of key tiles
share one [128, 1024] PSUM slot so exp runs as a single wide op per pair
(amortizes engine overheads ~25%).  Pairs with pt % 3 == 2 use a
one-instruction Schraudolph fast-exp on VectorE (scores*EA'+EB -> int16,
bitcast fp16, ~1.7% rms weight error, partially cancelling in
normalization); the rest exact Exp on ScalarE with the mean-matching bias
BMEAN so the two exp families agree in expectation.  PV accumulates
out[n, 4*97] with the exp tile as the stationary operand, lagging scores
by PVLAG key tiles so PE never waits on the exp engines; all PSUM
staging/epilogue copies ride VectorE (ScalarE is saturated by exp).
The softmax denominator rides column 96 of each 97-block via an
all-ones column in the V weights; normalization happens on the host in
gather_out.  PSUM budget: 2 score-pair slots (4 banks) + 2 PV
accumulators (2) + 2 projection slots (2) = 8 banks exactly.
"""

import sys

for _p in ("/opt/trn_rl_repo",):
    if _p not in sys.path:
        sys.path.insert(0, _p)

import numpy as np

B = 2
N = 4096
DIM = 768
H = 8
DH = 96
SCALE = DIM ** -0.5
NCORES = 8
VW = 2 * DH + 2  # 194: [v_h0 | ones | v_h1 | ones]
NT = N // 128    # 32 token tiles
NPAIR = NT // 2  # 16 key-tile pairs per window
NBLK = N // 512  # 8 blocks of 512
DT = DIM // 128  # 6 contraction tiles

_CACHE = {}
PVLAG = 6        # key tiles of score->PV lag
EXBUFS = 6       # exp pair tiles in flight
FILLER_MOD = 22  # spread the 23 filler projections over all ~506 PV pops
VLOOK = 6
KLOOK = 3

# Schraudolph fast-exp on DVE: bits_f16(exp(s*SCALE)) ~= int16(s*EA'+EB).
# EA' folds SCALE; EB = 1024*15 - 45 (bias tuned) + 0.5 (int16 convert
# truncates toward zero; inputs keep y positive).  Pairs with
# pt % 3 == SCHRAUD_PHASE use DVE; the rest exact exp on ACT with bias
# BMEAN = ln(mean Schraudolph/exact ratio) so the families' means match.
EA = 1024.0 / float(np.log(2.0)) * SCALE
EB = 1024.0 * 15 - 45.0 + 0.5
ASPLIT = 896     # pair-exp split: ACT exact on [0:896), DVE on [896:1024)
ASOLO = 0        # solo tiles: all-DVE Schraudolph (no ACT op)
BMEAN = float(np.log(1.00932))


def build_program(loop_iters=1, variant="full"):
    import concourse.tile as tile
    from concourse import bacc, mybir

    F16 = mybir.dt.float16
    F32 = mybir.dt.float32
    F8 = mybir.dt.float8e4
    I16 = mybir.dt.int16
    Exp = mybir.ActivationFunctionType.Exp
    Mult = mybir.AluOpType.mult
    Add = mybir.AluOpType.add
    DR = mybir.MatmulPerfMode.DoubleRow

    nc = bacc.Bacc("TRN2", target_bir_lowering=False, debug=False)
    xT_h = nc.declare_dram_parameter("xT", [DIM, N], F16, isOutput=False)
    wqk_h = nc.declare_dram_parameter("wqk", [DIM, 4 * DH], F16, isOutput=False)
    bqk_h = nc.declare_dram_parameter("bqk", [64, 8], F32, isOutput=False)
    wv_h = nc.declare_dram_parameter("wv", [DIM, VW], F16, isOutput=False)
    wvaug_h = nc.declare_dram_parameter("wvaug", [1, VW], F16, isOutput=False)
    # out[h, nw, p, a*97+c] = UNNORMALIZED attention numerator (c<96) and
    # softmax denominator (c=96) for head h, token nw*512 + a*128 + p.
    out_h = nc.declare_dram_parameter(
        "out", [2, NBLK, 128, 4 * 97], F32, isOutput=True
    )

    xT, wqk, bqk = xT_h.ap(), wqk_h.ap(), bqk_h.ap()
    wv, wvaug, out = wv_h.ap(), wvaug_h.ap(), out_h.ap()

    with tile.TileContext(nc) as tc:
        with (
            tc.tile_pool(name="const", bufs=1) as const,
            tc.tile_pool(name="work", bufs=3) as work,
            tc.tile_pool(name="pp", bufs=2, space="PSUM") as pp,
        ):
            # --- persistent SBUF tensors ---
            xt_sb = [
                const.tile([128, N], F16, name=f"xt{d}", tag=f"xt{d}")
                for d in range(DT)
            ]
            wqk_sb = [
                const.tile([128, 4 * DH], F16, name=f"wqksb{d}", tag=f"wqksb{d}")
                for d in range(DT)
            ]
            wv_sb = [
                const.tile([128, VW], F16, name=f"wvsb{d}", tag=f"wvsb{d}")
                for d in range(DT)
            ]
            wvaug_sb = const.tile([1, VW], F16, name="wvaug_sb")
            bqk_sb = const.tile([64, 8], F32, name="bqk_sb")
            ones_sb = const.tile([1, 128], F16, name="ones_sb")
            bm_sb = const.tile([128, 1], F32, name="bm_sb")
            # fp8 DoubleRow layout: [64 partitions, 2 k-subtiles, N tokens].
            # dh=96 splits 64+32 (not 48+48: PSUM reads must start at a
            # 32-aligned partition, so the epilogue reads pt[0:64]/pt[64:96]);
            # rows [32:64] of k-subtile 1 are zero (memset once below) and
            # contribute nothing to the 128-wide DoubleRow contraction.
            qkT8_sb = [
                const.tile([64, 2, N], F8, name=f"qkT8{j}", tag=f"qkT8{j}")
                for j in range(4)
            ]
            v_sb = const.tile([128, NT * VW], F16, name="v_sb")

            nc.sync.dma_start(out=bqk_sb, in_=bqk)
            nc.sync.dma_start(out=wvaug_sb, in_=wvaug)
            for d in range(DT):
                nc.sync.dma_start(out=wqk_sb[d], in_=wqk[d * 128:(d + 1) * 128, :])
            # xT arrives in column chunks, in the order the first attention
            # window consumes them; V weights slot in after the first chunk.
            for blk in range(NBLK):
                for d in range(DT):
                    nc.sync.dma_start(
                        out=xt_sb[d][:, blk * 512:(blk + 1) * 512],
                        in_=xT[d * 128:(d + 1) * 128, blk * 512:(blk + 1) * 512],
                    )
                if blk == 0:
                    for d in range(DT):
                        nc.sync.dma_start(
                            out=wv_sb[d], in_=wv[d * 128:(d + 1) * 128, :]
                        )
            nc.vector.memset(ones_sb, 1.0)
            nc.vector.memset(bm_sb, BMEAN)
            for j in range(4):
                nc.gpsimd.memset(qkT8_sb[j][32:64, 1, :], 0)

            qk_done = set()
            v_done = set()

            def ensure_qk(j, blk):
                # qkT8[j][:, ko, blk] = fp8((wqk[:, j] block).T @ xT[:, blk]
                #                           + bias half ko), natural scale
                if (j, blk) in qk_done:
                    return
                qk_done.add((j, blk))
                pt = pp.tile([DH, 512], F32, tag="pj", name="pt", bufs=2)
                for d in range(DT):
                    nc.tensor.matmul(
                        pt,
                        lhsT=wqk_sb[d][:, j * DH:(j + 1) * DH],
                        rhs=xt_sb[d][:, blk * 512:(blk + 1) * 512],
                        start=(d == 0),
                        stop=(d == DT - 1),
                    )
                nc.vector.tensor_scalar_add(
                    out=qkT8_sb[j][:, 0, blk * 512:(blk + 1) * 512],
                    in0=pt[0:64, :],
                    scalar1=bqk_sb[0:64, 2 * j:2 * j + 1],
                )
                nc.vector.tensor_scalar_add(
                    out=qkT8_sb[j][0:32, 1, blk * 512:(blk + 1) * 512],
                    in0=pt[64:96, :],
                    scalar1=bqk_sb[0:32, 2 * j + 1:2 * j + 2],
                )

            def ensure_v(t):
                if t in v_done:
                    return
                v_done.add(t)
                pv = pp.tile([128, VW], F32, tag="pj", name="pv", bufs=2)
                for d in range(DT):
                    nc.tensor.matmul(
                        pv,
                        lhsT=xt_sb[d][:, t * 128:(t + 1) * 128],
                        rhs=wv_sb[d],
                        start=(d == 0),
                        stop=False,
                    )
                nc.tensor.matmul(
                    pv,
                    lhsT=ones_sb,
                    rhs=wvaug_sb,
                    start=False,
                    stop=True,
                )
                nc.vector.tensor_copy(out=v_sb[:, t * VW:(t + 1) * VW], in_=pv)

            # filler: projection units to slip into PE slack inside the
            # exp-bound attention stream, ordered by deadline.
            filler = []
            for b in range(1, NBLK):
                filler.append((0, b))       # q_h0 blk b: before window (0, b)
                filler.append((3, b - 1))   # k_h1: all before head 1
            filler.append((3, NBLK - 1))
            for b in range(NBLK):
                filler.append((1, b))       # q_h1 blk b: before window (1, b)
            fill_state = {"i": 0, "tick": 0}

            def pop_filler():
                fill_state["tick"] += 1
                if fill_state["tick"] % FILLER_MOD == 0 and fill_state["i"] < len(filler):
                    j, b = filler[fill_state["i"]]
                    fill_state["i"] += 1
                    ensure_qk(j, b)

            # software pipeline: PV lags scores by PVLAG key tiles and the
            # queue carries ACROSS window boundaries (pva is double-buffered
            # and the epilogue is only a staging copy + DMA), so the pipeline
            # never drains mid-kernel.
            pending = []
            pending_noexp = []

            def emit_pv(hh, ww, pva_, mt, exa, exb, base, split):
                if variant != "nopv":
                    for ns in range(4):
                        c = base + ns * 128
                        lhsT = (exa[:, c:c + 128] if c < split
                                else exb[:, c - split:c - split + 128])
                        nc.tensor.matmul(
                            pva_[:, ns * 97:ns * 97 + 97],
                            lhsT=lhsT,
                            rhs=v_sb[:, mt * VW + hh * 97:mt * VW + hh * 97 + 97],
                            # PSUM start zeroing is bank-granular (2KB): the
                            # first matmul's start=True zeroes the whole
                            # (bank-aligned) pva slot before ns=1..3 land.
                            start=(mt == 0 and ns == 0),
                            stop=(mt == NT - 1 and ns == 3),
                            skip_group_check=True,
                        )
                if mt == NT - 1:
                    # window ww finished accumulating: stage + DMA it out
                    ob = work.tile([128, 4 * 97], F32, tag="ob", name="ob",
                                   bufs=3)
                    nc.vector.tensor_copy(out=ob, in_=pva_[:, :4 * 97])
                    nc.sync.dma_start(out=out[hh, ww], in_=ob)

            def attn_nw_stream():
                # Flat stream over all (h, nw, mt): score tiles rotate over
                # five single-bank PSUM slots inside one [128, 2560] tile
                # (range-based dep tracking gives 5-deep slot rotation, which
                # the 2-pair-slot scheme could not fit in 8 banks).  Slots
                # 0+1 and 2+3 form exp PAIRS (one wide op per engine), slot
                # 4 is exp'd solo.  Groups run freely across windows.
                state = {"pva": None, "sc": None}

                def one_tile(idx, h, nw, mt):
                    p = idx % 5
                    if mt == 0:
                        ensure_qk(h, nw)
                        state["pva"] = pp.tile([128, 512], F32, tag="pva",
                                               name="pva", bufs=1)
                    pva = state["pva"]
                    for b in range(mt * 128 // 512 + 1):
                        ensure_qk(2 + h, b)
                    if len(pending) >= PVLAG:
                        emit_pv(*pending.pop(0))
                        pop_filler()
                    if p == 0 or p == 2:
                        state["sc"] = pp.tile(
                            [128, 1024], F32, name="sc",
                            tag=("scA" if p == 0 else "scB"), bufs=1)
                    elif p == 4:
                        state["sc"] = pp.tile([128, 512], F32, name="sc",
                                              tag="scS", bufs=1)
                    sc = state["sc"]
                    sub = p % 2 if p < 4 else 0
                    nc.tensor.matmul(
                        sc[:, sub * 512:(sub + 1) * 512],
                        lhsT=qkT8_sb[2 + h][:, :, mt * 128:(mt + 1) * 128],
                        rhs=qkT8_sb[h][:, :, nw * 512:(nw + 1) * 512],
                        start=True,
                        stop=True,
                        perf_mode=DR,
                    )
                    pending_noexp.append((h, nw, pva, mt))
                    # exp after the 2nd member of a pair (p 1/3) or solo (4)
                    if p in (1, 3):
                        width, split = 1024, ASPLIT
                    elif p == 4:
                        width, split = 512, ASOLO  # all-DVE solo
                    else:
                        width = None
                    if width is not None:
                        exB = work.tile([128, width - split], I16,
                                        tag=f"exB{p}", name="exB", bufs=EXBUFS)
                        if split:
                            exA = work.tile([128, split], F16, tag=f"exA{p}",
                                            name="exA", bufs=EXBUFS)
                            nc.scalar.activation(
                                out=(exA if variant != "noexp" else exA[:, :8]),
                                in_=(sc[:, :split] if variant != "noexp"
                                     else sc[:, :8]),
                                func=Exp, bias=bm_sb, scale=SCALE)
                        else:
                            exA = None
                        nc.vector.tensor_scalar(
                            out=(exB if variant != "noexp" else exB[:, :8]),
                            in0=(sc[:, split:width] if variant != "noexp"
                                 else sc[:, split:split + 8]),
                            scalar1=EA, scalar2=EB, op0=Mult, op1=Add)
                        exb16 = exB.bitcast(F16)
                        # attach exp tiles to the 1-2 tiles of this exp op
                        n_t = 2 if width == 1024 else 1
                        for i in range(n_t):
                            hh, ww, pv_, mtt = pending_noexp.pop(0)
                            pending.append(
                                (hh, ww, pv_, mtt, exA, exb16,
                                 i * 512, split))
                    # look-ahead projections/V AFTER the exp so their engine
                    # tails don't delay the exp delivery
                    for b in range(min(mt + KLOOK, NT - 1) * 128 // 512 + 1):
                        ensure_qk(2 + h, b)
                    for t in range(mt, min(mt + VLOOK, NT)):
                        ensure_v(t)

                idx = 0
                for h in range(2):
                    for nw in range(NBLK):
                        for mt in range(NT):
                            one_tile(idx, h, nw, mt)
                            idx += 1

            # Emission order tuned for overlap: head-0 q/k projection and V
            # first, then attention for head 0 with head-1 projections
            # slipped in between the first windows.
            def body(_i=None):
                qk_done.clear()
                v_done.clear()
                fill_state["i"] = 0
                fill_state["tick"] = 0
                pending.clear()
                pending_noexp.clear()
                attn_nw_stream()
                for p in pending:
                    emit_pv(*p)
                    pop_filler()
                pending.clear()
                # backstop: anything the filler didn't reach
                for j, b in filler:
                    ensure_qk(j, b)

            if loop_iters == 1:
                body()
            else:
                with tc.For_i(0, loop_iters, 1) as _i:
                    body(_i)

    nc.compile()
    return nc


def get_program(loop_iters=1, variant="full"):
    key = ("nc", loop_iters, variant)
    if key not in _CACHE:
        _CACHE[key] = build_program(loop_iters, variant)
    return _CACHE[key]


def make_in_maps(x, W_qkv, b_qkv):
    x = np.asarray(x, np.float32)
    W = np.asarray(W_qkv, np.float32)
    b = np.asarray(b_qkv, np.float32)
    Wq, Wk, Wv = W[:, :DIM], W[:, DIM:2 * DIM], W[:, 2 * DIM:]
    bq, bk, bv = b[:DIM], b[DIM:2 * DIM], b[2 * DIM:]

    in_maps = []
    for c in range(NCORES):
        bb, hp = divmod(c, 4)
        h0 = 2 * hp
        s = slice(h0 * DH, (h0 + 1) * DH)
        s1 = slice((h0 + 1) * DH, (h0 + 2) * DH)
        xT = np.ascontiguousarray(x[bb].T).astype(np.float16)
        wqk = np.concatenate(
            [Wq[:, s], Wq[:, s1], Wk[:, s], Wk[:, s1]], axis=1
        ).astype(np.float16)
        # bias halves (64+32 dh split): col 2j = b_j[0:64]; col 2j+1
        # rows 0-31 = b_j[64:96]
        bj = np.stack([bq[s], bq[s1], bk[s], bk[s1]], axis=0)   # [4, 96]
        bqk = np.zeros((64, 8), np.float32)
        for j in range(4):
            bqk[:, 2 * j] = bj[j, :64]
            bqk[:32, 2 * j + 1] = bj[j, 64:]
        wv = np.zeros((DIM, VW), np.float16)
        wv[:, 0:DH] = Wv[:, s].astype(np.float16)
        wv[:, DH + 1:2 * DH + 1] = Wv[:, s1].astype(np.float16)
        wvaug = np.zeros((1, VW), np.float16)
        wvaug[0, 0:DH] = bv[s].astype(np.float16)
        wvaug[0, DH] = 1.0
        wvaug[0, DH + 1:2 * DH + 1] = bv[s1].astype(np.float16)
        wvaug[0, 2 * DH + 1] = 1.0
        in_maps.append(
            {"xT": xT, "wqk": wqk, "bqk": bqk, "wv": wv, "wvaug": wvaug}
        )
    return in_maps


def gather_out(results):
    out = np.empty((B, N, DIM), np.float32)
    for c in range(NCORES):
        bb, hp = divmod(c, 4)
        o = np.asarray(results[c]["out"], np.float32)  # [2, NBLK, 128, 4*97]
        # token n = nw*512 + a*128 + p lives at o[h, nw, p, a*97:(a+1)*97];
        # col 96 of each 97-block is the softmax denominator
        o = o.reshape(2, NBLK, 128, 4, 97).transpose(0, 1, 3, 2, 4)
        o = (o[..., :DH] / o[..., DH:]).reshape(2, N, DH)
        out[bb, :, (2 * hp) * DH:(2 * hp + 1) * DH] = o[0]
        out[bb, :, (2 * hp + 1) * DH:(2 * hp + 2) * DH] = o[1]
    return out


def run(x, W_qkv, b_qkv, trace=False, **kw):
    from concourse.bass_utils import run_bass_kernel_spmd

    nc = get_program()
    in_maps = make_in_maps(x, W_qkv, b_qkv)
    res = run_bass_kernel_spmd(nc, in_maps, list(range(NCORES)), trace=trace, **kw)
    return gather_out(res.results), res


def kernel(x, W_qkv, b_qkv):
    out, _ = run(x, W_qkv, b_qkv)
    return out


# revision 14
# speedup vs baseline: 1.1790x; 1.1790x over previous
"""Multi-head attention (B=2, N=4096, D=768, H=8) on 8 trn2 NeuronCores.

Sharding: core c handles batch b = c//4 and head-pair hp = c%4 (heads 2hp,
2hp+1).  Each core computes the qkv projection for its 2 heads plus full
4096x4096 attention for them; no cross-core communication.

Device-side layout (per core):
  xT    [768, 4096] fp16   x[b] transposed (host-prepped)
  wqk   [768, 384]  fp16   [Wq_h0 | Wq_h1 | Wk_h0 | Wk_h1]  (UNSCALED)
  bqk   [64, 8]     fp32   bias halves: col 2j=b_j[0:64], col 2j+1=b_j[64:96]
  wv    [768, 194]  fp16   [Wv_h0 | 0 | Wv_h1 | 0]
  wvaug [1, 194]    fp16   [bv_h0 | 1 | bv_h1 | 1]  (ones row of aug x)
  out   [2, 8, 128, 388] fp32 unnormalized numerators + denominators

Algorithm: q/k are produced at natural scale (softmax SCALE applied inside
the exp) and stored as fp8e4m3 in DoubleRow layout [64, 2, N] (dh=96 zero-padded
to 128 and split as 64 partitions x 2 interleaved k-subtiles).  Scores run on
the PE in fp8 DoubleRow perf mode at 0.5 cycles/row -- half the fp16 cost.
V stays fp16 (fp8 V would blow the 2e-2 error budget).

Per query window of 512 tokens, scores are computed transposed
S^T[m, n] = kT8.T @ qT8 one 128-key tile at a time; PAIRS of key tiles
share one [128, 1024] PSUM slot so exp runs as a single wide op per pair
(amortizes engine overheads ~25%).  Pairs with pt % 3 == 2 use a
one-instruction Schraudolph fast-exp on VectorE (scores*EA'+EB -> int16,
bitcast fp16, ~1.7% rms weight error, partially cancelling in
normalization); the rest exact Exp on ScalarE with the mean-matching bias
BMEAN so the two exp families agree in expectation.  PV accumulates
out[n, 4*97] with the exp tile as the stationary operand, lagging scores
by PVLAG key tiles so PE never waits on the exp engines; all PSUM
staging/epilogue copies ride VectorE (ScalarE is saturated by exp).
The softmax denominator rides column 96 of each 97-block via an
all-ones column in the V weights; normalization happens on the host in
gather_out.  PSUM budget: 2 score-pair slots (4 banks) + 2 PV
accumulators (2) + 2 projection slots (2) = 8 banks exactly.
"""

import sys

for _p in ("/opt/trn_rl_repo",):
    if _p not in sys.path:
        sys.path.insert(0, _p)

import numpy as np

B = 2
N = 4096
DIM = 768
H = 8
DH = 96
SCALE = DIM ** -0.5
NCORES = 8
VW = 2 * DH + 2  # 194: [v_h0 | ones | v_h1 | ones]
NT = N // 128    # 32 token tiles
NPAIR = NT // 2  # 16 key-tile pairs per window
NBLK = N // 512  # 8 blocks of 512
DT = DIM // 128  # 6 contraction tiles

_CACHE = {}
PVLAG = 6        # key tiles of score->PV lag
EXBUFS = 6       # exp pair tiles in flight
FILLER_MOD = 22  # spread the 23 filler projections over all ~506 PV pops
VLOOK = 6
KLOOK = 3

# Schraudolph fast-exp on DVE: bits_f16(exp(s*SCALE)) ~= int16(s*EA'+EB).
# EA' folds SCALE; EB = 1024*15 - 45 (bias tuned) + 0.5 (int16 convert
# truncates toward zero; inputs keep y positive).  Pairs with
# pt % 3 == SCHRAUD_PHASE use DVE; the rest exact exp on ACT with bias
# BMEAN = ln(mean Schraudolph/exact ratio) so the families' means match.
EA = 1024.0 / float(np.log(2.0)) * SCALE
EB = 1024.0 * 15 - 45.0 + 0.5
ASPLIT = 896     # pair-exp split: ACT exact on [0:896), DVE on [896:1024)
ASOLO = 0        # solo tiles: all-DVE Schraudolph (no ACT op)
BMEAN = float(np.log(1.00932))


def build_program(loop_iters=1, variant="full"):
    import concourse.tile as tile
    from concourse import bacc, mybir

    F16 = mybir.dt.float16
    F32 = mybir.dt.float32
    F8 = mybir.dt.float8e4
    I16 = mybir.dt.int16
    Exp = mybir.ActivationFunctionType.Exp
    Mult = mybir.AluOpType.mult
    Add = mybir.AluOpType.add
    DR = mybir.MatmulPerfMode.DoubleRow

    nc = bacc.Bacc("TRN2", target_bir_lowering=False, debug=False)
    xT_h = nc.declare_dram_parameter("xT", [DIM, N], F16, isOutput=False)
    wqk_h = nc.declare_dram_parameter("wqk", [DIM, 4 * DH], F16, isOutput=False)
    bqk_h = nc.declare_dram_parameter("bqk", [64, 8], F32, isOutput=False)
    bqk96_h = nc.declare_dram_parameter("bqk96", [DH, 4], F32, isOutput=False)
    wv_h = nc.declare_dram_parameter("wv", [DIM, VW], F16, isOutput=False)
    wvaug_h = nc.declare_dram_parameter("wvaug", [1, VW], F16, isOutput=False)
    # out[h, nw, p, a*97+c] = UNNORMALIZED attention numerator (c<96) and
    # softmax denominator (c=96) for head h, token nw*512 + a*128 + p.
    out_h = nc.declare_dram_parameter(
        "out", [2, NBLK, 128, 4 * 97], F32, isOutput=True
    )

    xT, wqk, bqk = xT_h.ap(), wqk_h.ap(), bqk_h.ap()
    bqk96 = bqk96_h.ap()
    wv, wvaug, out = wv_h.ap(), wvaug_h.ap(), out_h.ap()

    with tile.TileContext(nc) as tc:
        with (
            tc.tile_pool(name="const", bufs=1) as const,
            tc.tile_pool(name="work", bufs=3) as work,
            tc.tile_pool(name="pp", bufs=2, space="PSUM") as pp,
        ):
            # --- persistent SBUF tensors ---
            xt_sb = [
                const.tile([128, N], F16, name=f"xt{d}", tag=f"xt{d}")
                for d in range(DT)
            ]
            wqk_sb = [
                const.tile([128, 4 * DH], F16, name=f"wqksb{d}", tag=f"wqksb{d}")
                for d in range(DT)
            ]
            wv_sb = [
                const.tile([128, VW], F16, name=f"wvsb{d}", tag=f"wvsb{d}")
                for d in range(DT)
            ]
            wvaug_sb = const.tile([1, VW], F16, name="wvaug_sb")
            bqk_sb = const.tile([64, 8], F32, name="bqk_sb")
            bqk96_sb = const.tile([DH, 4], F32, name="bqk96_sb")
            ones_sb = const.tile([1, 128], F16, name="ones_sb")
            bm_sb = const.tile([128, 1], F32, name="bm_sb")
            # fp8 DoubleRow layout: [64 partitions, 2 k-subtiles, N tokens].
            # dh=96 splits 64+32 (not 48+48: PSUM reads must start at a
            # 32-aligned partition, so the epilogue reads pt[0:64]/pt[64:96]);
            # rows [32:64] of k-subtile 1 are zero (memset once below) and
            # contribute nothing to the 128-wide DoubleRow contraction.
            # head 0 (j=0 q, j=2 k): fp8 DoubleRow; head 1 (j=1, 3): fp16.
            # Mixing precision per head halves the fp8 quantization noise in
            # the overall output norm while keeping one copy of each tensor.
            qkT8_sb = {
                j: const.tile([64, 2, N], F8, name=f"qkT8{j}", tag=f"qkT8{j}")
                for j in (0, 2)
            }
            qkT16_sb = {
                j: const.tile([DH, N], F16, name=f"qkT16{j}", tag=f"qkT16{j}")
                for j in (1, 3)
            }
            v_sb = const.tile([128, NT * VW], F16, name="v_sb")

            nc.sync.dma_start(out=bqk_sb, in_=bqk)
            nc.sync.dma_start(out=bqk96_sb, in_=bqk96)
            nc.sync.dma_start(out=wvaug_sb, in_=wvaug)
            for d in range(DT):
                nc.sync.dma_start(out=wqk_sb[d], in_=wqk[d * 128:(d + 1) * 128, :])
            # xT arrives in column chunks, in the order the first attention
            # window consumes them; V weights slot in after the first chunk.
            for blk in range(NBLK):
                for d in range(DT):
                    nc.sync.dma_start(
                        out=xt_sb[d][:, blk * 512:(blk + 1) * 512],
                        in_=xT[d * 128:(d + 1) * 128, blk * 512:(blk + 1) * 512],
                    )
                if blk == 0:
                    for d in range(DT):
                        nc.sync.dma_start(
                            out=wv_sb[d], in_=wv[d * 128:(d + 1) * 128, :]
                        )
            nc.vector.memset(ones_sb, 1.0)
            nc.vector.memset(bm_sb, BMEAN)
            for j in (0, 2):
                nc.gpsimd.memset(qkT8_sb[j][32:64, 1, :], 0)

            qk_done = set()
            v_done = set()

            def ensure_qk(j, blk):
                # qkT8[j][:, ko, blk] = fp8((wqk[:, j] block).T @ xT[:, blk]
                #                           + bias half ko), natural scale
                if (j, blk) in qk_done:
                    return
                qk_done.add((j, blk))
                pt = pp.tile([DH, 512], F32, tag="pj", name="pt", bufs=2)
                for d in range(DT):
                    nc.tensor.matmul(
                        pt,
                        lhsT=wqk_sb[d][:, j * DH:(j + 1) * DH],
                        rhs=xt_sb[d][:, blk * 512:(blk + 1) * 512],
                        start=(d == 0),
                        stop=(d == DT - 1),
                    )
                if j in (0, 2):
                    nc.vector.tensor_scalar_add(
                        out=qkT8_sb[j][:, 0, blk * 512:(blk + 1) * 512],
                        in0=pt[0:64, :],
                        scalar1=bqk_sb[0:64, 2 * j:2 * j + 1],
                    )
                    nc.vector.tensor_scalar_add(
                        out=qkT8_sb[j][0:32, 1, blk * 512:(blk + 1) * 512],
                        in0=pt[64:96, :],
                        scalar1=bqk_sb[0:32, 2 * j + 1:2 * j + 2],
                    )
                else:
                    nc.vector.tensor_scalar_add(
                        out=qkT16_sb[j][:, blk * 512:(blk + 1) * 512],
                        in0=pt,
                        scalar1=bqk96_sb[:, j:j + 1],
                    )

            def ensure_v(t):
                if t in v_done:
                    return
                v_done.add(t)
                pv = pp.tile([128, VW], F32, tag="pj", name="pv", bufs=2)
                for d in range(DT):
                    nc.tensor.matmul(
                        pv,
                        lhsT=xt_sb[d][:, t * 128:(t + 1) * 128],
                        rhs=wv_sb[d],
                        start=(d == 0),
                        stop=False,
                    )
                nc.tensor.matmul(
                    pv,
                    lhsT=ones_sb,
                    rhs=wvaug_sb,
                    start=False,
                    stop=True,
                )
                nc.vector.tensor_copy(out=v_sb[:, t * VW:(t + 1) * VW], in_=pv)

            # filler: projection units to slip into PE slack inside the
            # exp-bound attention stream, ordered by deadline.
            filler = []
            for b in range(1, NBLK):
                filler.append((0, b))       # q_h0 blk b: before window (0, b)
                filler.append((3, b - 1))   # k_h1: all before head 1
            filler.append((3, NBLK - 1))
            for b in range(NBLK):
                filler.append((1, b))       # q_h1 blk b: before window (1, b)
            fill_state = {"i": 0, "tick": 0}

            def pop_filler():
                fill_state["tick"] += 1
                if fill_state["tick"] % FILLER_MOD == 0 and fill_state["i"] < len(filler):
                    j, b = filler[fill_state["i"]]
                    fill_state["i"] += 1
                    ensure_qk(j, b)

            # software pipeline: PV lags scores by PVLAG key tiles and the
            # queue carries ACROSS window boundaries (pva is double-buffered
            # and the epilogue is only a staging copy + DMA), so the pipeline
            # never drains mid-kernel.
            pending = []
            pending_noexp = []

            def emit_pv(hh, ww, pva_, mt, exa, exb, base, split):
                if variant != "nopv":
                    for ns in range(4):
                        c = base + ns * 128
                        lhsT = (exa[:, c:c + 128] if c < split
                                else exb[:, c - split:c - split + 128])
                        nc.tensor.matmul(
                            pva_[:, ns * 97:ns * 97 + 97],
                            lhsT=lhsT,
                            rhs=v_sb[:, mt * VW + hh * 97:mt * VW + hh * 97 + 97],
                            # PSUM start zeroing is bank-granular (2KB): the
                            # first matmul's start=True zeroes the whole
                            # (bank-aligned) pva slot before ns=1..3 land.
                            start=(mt == 0 and ns == 0),
                            stop=(mt == NT - 1 and ns == 3),
                            skip_group_check=True,
                        )
                if mt == NT - 1:
                    # window ww finished accumulating: stage + DMA it out
                    ob = work.tile([128, 4 * 97], F32, tag="ob", name="ob",
                                   bufs=3)
                    nc.vector.tensor_copy(out=ob, in_=pva_[:, :4 * 97])
                    nc.sync.dma_start(out=out[hh, ww], in_=ob)

            def attn_nw_stream():
                # Flat stream over all (h, nw, mt): score tiles rotate over
                # five single-bank PSUM slots inside one [128, 2560] tile
                # (range-based dep tracking gives 5-deep slot rotation, which
                # the 2-pair-slot scheme could not fit in 8 banks).  Slots
                # 0+1 and 2+3 form exp PAIRS (one wide op per engine), slot
                # 4 is exp'd solo.  Groups run freely across windows.
                state = {"pva": None, "sc": None}

                def one_tile(idx, h, nw, mt):
                    p = idx % 5
                    if mt == 0:
                        ensure_qk(h, nw)
                        state["pva"] = pp.tile([128, 512], F32, tag="pva",
                                               name="pva", bufs=1)
                    pva = state["pva"]
                    for b in range(mt * 128 // 512 + 1):
                        ensure_qk(2 + h, b)
                    if len(pending) >= PVLAG:
                        emit_pv(*pending.pop(0))
                        pop_filler()
                    if p == 0 or p == 2:
                        state["sc"] = pp.tile(
                            [128, 1024], F32, name="sc",
                            tag=("scA" if p == 0 else "scB"), bufs=1)
                    elif p == 4:
                        state["sc"] = pp.tile([128, 512], F32, name="sc",
                                              tag="scS", bufs=1)
                    sc = state["sc"]
                    sub = p % 2 if p < 4 else 0
                    if h == 0:
                        nc.tensor.matmul(
                            sc[:, sub * 512:(sub + 1) * 512],
                            lhsT=qkT8_sb[2][:, :, mt * 128:(mt + 1) * 128],
                            rhs=qkT8_sb[0][:, :, nw * 512:(nw + 1) * 512],
                            start=True,
                            stop=True,
                            perf_mode=DR,
                        )
                    else:
                        nc.tensor.matmul(
                            sc[:, sub * 512:(sub + 1) * 512],
                            lhsT=qkT16_sb[3][:, mt * 128:(mt + 1) * 128],
                            rhs=qkT16_sb[1][:, nw * 512:(nw + 1) * 512],
                            start=True,
                            stop=True,
                        )
                    pending_noexp.append((h, nw, pva, mt))
                    # exp after the 2nd member of a pair (p 1/3) or solo (4)
                    if p in (1, 3):
                        width, split = 1024, ASPLIT
                    elif p == 4:
                        width, split = 512, ASOLO  # all-DVE solo
                    else:
                        width = None
                    if width is not None:
                        if variant == "allact":
                            exA = work.tile([128, width], F16, tag=f"exA{p}",
                                            name="exA", bufs=EXBUFS)
                            nc.scalar.activation(out=exA, in_=sc[:, :width],
                                                 func=Exp, bias=bm_sb,
                                                 scale=SCALE)
                            n_t = 2 if width == 1024 else 1
                            for i in range(n_t):
                                hh, ww, pv_, mtt = pending_noexp.pop(0)
                                pending.append((hh, ww, pv_, mtt, exA, exA,
                                                i * 512, width))
                            width = None
                    if width is not None:
                        exB = work.tile([128, width - split], I16,
                                        tag=f"exB{p}", name="exB", bufs=EXBUFS)
                        if split:
                            exA = work.tile([128, split], F16, tag=f"exA{p}",
                                            name="exA", bufs=EXBUFS)
                            nc.scalar.activation(
                                out=(exA if variant != "noexp" else exA[:, :8]),
                                in_=(sc[:, :split] if variant != "noexp"
                                     else sc[:, :8]),
                                func=Exp, bias=bm_sb, scale=SCALE)
                        else:
                            exA = None
                        nc.vector.tensor_scalar(
                            out=(exB if variant != "noexp" else exB[:, :8]),
                            in0=(sc[:, split:width] if variant != "noexp"
                                 else sc[:, split:split + 8]),
                            scalar1=EA, scalar2=EB, op0=Mult, op1=Add)
                        exb16 = exB.bitcast(F16)
                        # attach exp tiles to the 1-2 tiles of this exp op
                        n_t = 2 if width == 1024 else 1
                        for i in range(n_t):
                            hh, ww, pv_, mtt = pending_noexp.pop(0)
                            pending.append(
                                (hh, ww, pv_, mtt, exA, exb16,
                                 i * 512, split))
                    # look-ahead projections/V AFTER the exp so their engine
                    # tails don't delay the exp delivery
                    for b in range(min(mt + KLOOK, NT - 1) * 128 // 512 + 1):
                        ensure_qk(2 + h, b)
                    for t in range(mt, min(mt + VLOOK, NT)):
                        ensure_v(t)

                idx = 0
                for h in range(2):
                    for nw in range(NBLK):
                        for mt in range(NT):
                            one_tile(idx, h, nw, mt)
                            idx += 1

            # Emission order tuned for overlap: head-0 q/k projection and V
            # first, then attention for head 0 with head-1 projections
            # slipped in between the first windows.
            def body(_i=None):
                qk_done.clear()
                v_done.clear()
                fill_state["i"] = 0
                fill_state["tick"] = 0
                pending.clear()
                pending_noexp.clear()
                attn_nw_stream()
                for p in pending:
                    emit_pv(*p)
                    pop_filler()
                pending.clear()
                # backstop: anything the filler didn't reach
                for j, b in filler:
                    ensure_qk(j, b)

            if loop_iters == 1:
                body()
            else:
                with tc.For_i(0, loop_iters, 1) as _i:
                    body(_i)

    nc.compile()
    return nc


def get_program(loop_iters=1, variant="full"):
    key = ("nc", loop_iters, variant)
    if key not in _CACHE:
        _CACHE[key] = build_program(loop_iters, variant)
    return _CACHE[key]


def make_in_maps(x, W_qkv, b_qkv):
    x = np.asarray(x, np.float32)
    W = np.asarray(W_qkv, np.float32)
    b = np.asarray(b_qkv, np.float32)
    Wq, Wk, Wv = W[:, :DIM], W[:, DIM:2 * DIM], W[:, 2 * DIM:]
    bq, bk, bv = b[:DIM], b[DIM:2 * DIM], b[2 * DIM:]

    in_maps = []
    for c in range(NCORES):
        bb, hp = divmod(c, 4)
        h0 = 2 * hp
        s = slice(h0 * DH, (h0 + 1) * DH)
        s1 = slice((h0 + 1) * DH, (h0 + 2) * DH)
        xT = np.ascontiguousarray(x[bb].T).astype(np.float16)
        wqk = np.concatenate(
            [Wq[:, s], Wq[:, s1], Wk[:, s], Wk[:, s1]], axis=1
        ).astype(np.float16)
        # bias halves (64+32 dh split): col 2j = b_j[0:64]; col 2j+1
        # rows 0-31 = b_j[64:96]
        bj = np.stack([bq[s], bq[s1], bk[s], bk[s1]], axis=0)   # [4, 96]
        bqk = np.zeros((64, 8), np.float32)
        for j in range(4):
            bqk[:, 2 * j] = bj[j, :64]
            bqk[:32, 2 * j + 1] = bj[j, 64:]
        bqk96 = np.ascontiguousarray(bj.T)                      # [96, 4]
        wv = np.zeros((DIM, VW), np.float16)
        wv[:, 0:DH] = Wv[:, s].astype(np.float16)
        wv[:, DH + 1:2 * DH + 1] = Wv[:, s1].astype(np.float16)
        wvaug = np.zeros((1, VW), np.float16)
        wvaug[0, 0:DH] = bv[s].astype(np.float16)
        wvaug[0, DH] = 1.0
        wvaug[0, DH + 1:2 * DH + 1] = bv[s1].astype(np.float16)
        wvaug[0, 2 * DH + 1] = 1.0
        in_maps.append(
            {"xT": xT, "wqk": wqk, "bqk": bqk, "bqk96": bqk96,
             "wv": wv, "wvaug": wvaug}
        )
    return in_maps


def gather_out(results):
    out = np.empty((B, N, DIM), np.float32)
    for c in range(NCORES):
        bb, hp = divmod(c, 4)
        o = np.asarray(results[c]["out"], np.float32)  # [2, NBLK, 128, 4*97]
        # token n = nw*512 + a*128 + p lives at o[h, nw, p, a*97:(a+1)*97];
        # col 96 of each 97-block is the softmax denominator
        o = o.reshape(2, NBLK, 128, 4, 97).transpose(0, 1, 3, 2, 4)
        o = (o[..., :DH] / o[..., DH:]).reshape(2, N, DH)
        out[bb, :, (2 * hp) * DH:(2 * hp + 1) * DH] = o[0]
        out[bb, :, (2 * hp + 1) * DH:(2 * hp + 2) * DH] = o[1]
    return out


def run(x, W_qkv, b_qkv, trace=False, **kw):
    from concourse.bass_utils import run_bass_kernel_spmd

    nc = get_program()
    in_maps = make_in_maps(x, W_qkv, b_qkv)
    res = run_bass_kernel_spmd(nc, in_maps, list(range(NCORES)), trace=trace, **kw)
    return gather_out(res.results), res


def kernel(x, W_qkv, b_qkv):
    out, _ = run(x, W_qkv, b_qkv)
    return out
